# revision 11
# baseline (speedup 1.0000x reference)
"""GCN (4-layer + MLP head/tail) on 8 TRN2 NeuronCores.

Design:
  - Nodes padded to 50008 = 8 shards of SH=6251; core c owns dst shard c.
  - Node features for gathers live in 4 SBUF "window" tables [128, 6252] f16:
    window w rows 0-63 = shard w nodes (1 feat/partition), rows 64-127 =
    shard w+4.  Col 0 of each window is a zero sentinel.
  - Aggregation = per-(window,stream) k-step prefix gathers (indirect_copy,
    the one natively supported SBUF gather; table <= 16KB/partition) into
    per-window accumulators, degree-sorted so step k covers a prefix.
  - Per-window merge = one permuting indirect_copy back to canonical order,
    summed into a [128, SH] master (top rows = shards 0-3 sums, bottom =
    shards 4-7).
  - The cross-half fold AND the GCN weight multiply fuse into one matmul
    with vertically stacked weights [W;W] (K=128).
  - dst-side deg_isqrt scaling commutes with the matmul -> applied per
    column on PSUM tiles (DVE), then ACT does bias+LeakyReLU.
  - Inter-layer exchange: DMA g-slice to DRAM + ncfw CC AllGather.
Falls back to a numpy implementation if the device path fails.
"""
import sys

sys.path.insert(0, "/opt/trn_rl_repo")

import numpy as np

N_NODES = 50000
N_EDGES = 800000
IN_CH = 128
HID = 64
OUT_CH = 2
NEG_SLOPE = 0.2
NCORES = 8
LAST_EXEC_NS = None
LAST_RES = None
SH = 6272                      # nodes per shard, mult of 32 (8*6272 = 50176)
NPAD = NCORES * SH
TCOL = SH + 2                  # table cols (even): sentinel col 0, nodes 1..SH
CH = 1024                      # indirect_copy output cap (elems/partition)
MMT = 512                      # matmul moving-dim tile


def _chunks(n, c=CH):
    out = []
    off = 0
    while off < n:
        out.append((off, min(c, n - off)))
        off += c
    return out


def _wrap2(top, bot, slots):
    """top/bot: 1-D uint16 lists (len <= slots*16) -> [128, slots] wrapped:
    groups 0-3 (partitions 0-63) hold `top`, groups 4-7 hold `bot`."""
    blk = np.zeros((128, slots), np.uint16)
    for arr, g0 in ((top, 0), (bot, 4)):
        a = np.zeros(slots * 16, np.uint16)
        a[: len(arr)] = arr
        a = a.reshape(slots, 16).T  # [16, slots]
        for g in range(g0, g0 + 4):
            blk[16 * g: 16 * g + 16] = a
    return blk


def _preprocess(edge_index):
    """Build the uniform gather schedule + per-core index blobs."""
    ei = np.asarray(edge_index)
    src = np.concatenate([ei[0].astype(np.int64), np.arange(N_NODES, dtype=np.int64)])
    dst = np.concatenate([ei[1].astype(np.int64), np.arange(N_NODES, dtype=np.int64)])
    deg = np.bincount(dst, minlength=N_NODES).astype(np.float32)
    dis = np.zeros(NPAD, np.float32)
    dis[:N_NODES] = 1.0 / np.sqrt(np.maximum(deg, 1.0))

    owner = dst // SH
    bkt = src // SH
    dloc = (dst - owner * SH).astype(np.int64)
    scol = (src - bkt * SH + 1).astype(np.uint16)  # 1-based; 0 = sentinel

    order = np.lexsort((dloc, bkt, owner))
    so, sb, sdl, scl = owner[order], bkt[order], dloc[order], scol[order]
    key = (so * 8 + sb) * SH + sdl
    starts = np.r_[True, np.diff(key) != 0]
    run_start = np.flatnonzero(starts)
    run_id = np.cumsum(starts) - 1
    k_rank = np.arange(len(key)) - run_start[run_id]
    lens = np.diff(np.r_[run_start, len(key)])

    degob = np.zeros(8 * 8 * SH, np.int32)
    degob[key[run_start]] = lens
    degob = degob.reshape(8, 8, SH)

    # per (core o, bucket b): degree-sorted order, inverse positions, A matrix
    pos = np.zeros((8, 8, SH), np.uint16)
    A = {}
    maxk = np.zeros((8, 8), np.int32)
    for o in range(8):
        for b in range(8):
            d = degob[o, b]
            pi = np.argsort(-d, kind="stable")
            p = np.empty(SH, np.uint16)
            p[pi] = np.arange(SH, dtype=np.uint16)
            pos[o, b] = p
            mk = int(d.max())
            maxk[o, b] = mk
            A[(o, b)] = np.zeros((max(mk, 1), SH), np.uint16)
    for o in range(8):
        mo = so == o
        for b in range(8):
            m = mo & (sb == b)
            A[(o, b)][k_rank[m], pos[o, b][sdl[m]]] = scl[m]

    # prefix counts n[o,b,k] = #{deg_ob > k}
    nob = {}
    for o in range(8):
        for b in range(8):
            d = degob[o, b]
            nob[(o, b)] = np.array([(d > k).sum() for k in range(maxk[o, b])],
                                   np.int32)

    # uniform schedule: list of sub-instructions
    # entry: dict(kind, w, off, n, so(slot off), sl(slots), k)
    sched = []
    slot_off = 0

    def add(kind, w, off, n, k=0):
        nonlocal slot_off
        sl = (n + 15) // 16
        sched.append(dict(kind=kind, w=w, off=off, n=n, k=k,
                          so=slot_off, sl=sl))
        slot_off += sl

    for w in range(4):
        kmax = 1
        for o in range(8):
            kmax = max(kmax, maxk[o, w], maxk[o, w + 4])
        # packed k-segments: (k, nk) sizes, chopped into 1024-col calls
        segs = []
        for k in range(kmax):
            nk = 0
            for o in range(8):
                for b in (w, w + 4):
                    nl = nob[(o, b)]
                    if k < len(nl):
                        nk = max(nk, int(nl[k]))
            if nk == 0:
                continue
            nk = min((nk + 31) // 32 * 32, SH)
            segs.append((k, nk))
        # chop the concatenated segment stream into CH-col calls
        cur = []   # pieces in this call: (k, step_off, n)
        used = 0
        for k, nk in segs:
            off = 0
            while off < nk:
                take = min(CH - used, nk - off, CH)
                cur.append((k, off, take))
                used += take
                off += take
                if used == CH:
                    sl = CH // 16
                    sched.append(dict(kind="kpack", w=w, pieces=list(cur),
                                      n=CH, so=slot_off, sl=sl))
                    slot_off += sl
                    cur, used = [], 0
        if used:
            un = (used + 31) // 32 * 32
            sl = un // 16
            sched.append(dict(kind="kpack", w=w, pieces=list(cur),
                              n=un, so=slot_off, sl=sl))
            slot_off += sl
        for off, n in _chunks(SH):
            add("merge", w, off, n, 0)
    nslot = slot_off

    # per-core index blobs
    blobs = []
    for o in range(8):
        blob = np.zeros((128, nslot), np.uint16)
        for e in sched:
            w = e["w"]
            if e["kind"] == "kpack":
                rows = []
                for b in (w, w + 4):
                    Ao = A[(o, b)]
                    parts = []
                    for k, off, n in e["pieces"]:
                        if k < Ao.shape[0]:
                            parts.append(Ao[k, off:off + n])
                        else:
                            parts.append(np.zeros(n, np.uint16))
                    rows.append(np.concatenate(parts) if parts else
                                np.zeros(0, np.uint16))
                blk = _wrap2(rows[0], rows[1], e["sl"])
            else:  # merge
                off, n = e["off"], e["n"]
                blk = _wrap2(pos[o, w][off:off + n],
                             pos[o, w + 4][off:off + n], e["sl"])
            blob[:, e["so"]: e["so"] + e["sl"]] = blk
        blobs.append(blob)

    return sched, nslot, blobs, dis


def _build_bass(sched, nslot):
    import concourse.bass as bass
    import concourse.mybir as mybir

    F16 = mybir.dt.float16
    F32 = mybir.dt.float32
    U16 = mybir.dt.uint16
    OP = mybir.AluOpType
    AF = mybir.ActivationFunctionType

    nc = bass.Bass(num_devices=NCORES)

    ext = {}
    for name, shape, dt in [
        ("xT", [128, SH], F16), ("idxall", [128, nslot], U16),
        ("d64", [64, SH], F16),
        ("w1", [128, 32], F16), ("w2", [32, 64], F16),
        ("wst", [128, 4, 64], F16), ("wf", [64, 2], F16),
        ("b1", [32, 1], F32), ("b2", [64, 1], F32),
        ("bc", [64, 4], F32), ("bf", [2, 1], F32), ("alp", [64, 1], F32),
    ]:
        ext[name] = nc.declare_dram_parameter(name, shape, dt, isOutput=False)
    out_ext = nc.declare_dram_parameter("outp", [2, SH], F16, isOutput=True)
    dbg_fc = nc.declare_dram_parameter("dbg_fc", [64, SH], F16, isOutput=True)
    dbg_mst = nc.declare_dram_parameter("dbg_mst", [128, SH], F16, isOutput=True)
    dbg_o1 = nc.declare_dram_parameter("dbg_o1", [64, SH], F16, isOutput=True)
    ag_in = nc.dram_tensor("ag_in", [64, SH], F16)
    ag_out = nc.dram_tensor("ag_out", [8, 64, SH], F16)

    from contextlib import ExitStack
    with ExitStack() as _st:
        def _sb(name, shape, dt):
            return _st.enter_context(nc.sbuf_tensor(name, shape, dt))

        idxs = _sb("idx_s", [128, nslot], U16)
        tab = _sb("tab_s", [128, 4, TCOL], F16)
        acc = _sb("acc_s", [128, 4, SH], F16)
        scr = _sb("scr_s", [128, 2, CH], F16)
        mrg = _sb("mrg_s", [128, SH], F16)
        mst = _sb("mst_s", [128, SH], F16)
        hA = _sb("hA_s", [64, SH], F16)
        hB = _sb("hB_s", [64, SH], F16)
        h0 = _sb("h0_s", [32, SH], F16)
        gbuf = _sb("g_s", [64, SH], F16)
        d64 = _sb("d64_s", [64, SH], F16)
        w1 = _sb("w1_s", [128, 32], F16)
        w2 = _sb("w2_s", [32, 64], F16)
        wst = _sb("wst_s", [128, 4, 64], F16)
        wf = _sb("wf_s", [64, 2], F16)
        b1 = _sb("b1_s", [32, 1], F32)
        b2 = _sb("b2_s", [64, 1], F32)
        bc = _sb("bc_s", [64, 4], F32)
        bf = _sb("bf_s", [2, 1], F32)
        alp = _sb("alp_s", [64, 1], F32)
        ps0 = _st.enter_context(nc.psum_tensor("ps0", [64, MMT], F32))
        ps1 = _st.enter_context(nc.psum_tensor("ps1", [64, MMT], F32))
        sd = _st.enter_context(nc.semaphore("sd"))
        gp = _st.enter_context(nc.semaphore("gp"))
        ve = _st.enter_context(nc.semaphore("ve"))
        pe = _st.enter_context(nc.semaphore("pe"))
        ac = _st.enter_context(nc.semaphore("ac"))
        cc = _st.enter_context(nc.semaphore("cc"))
        block = _st.enter_context(nc.Block())

        ps = [ps0, ps1]
        C = dict(sd=0, gp=0, ve=0, pe=0, ac=0, cc=0)
        S = dict(sd=sd, gp=gp, ve=ve, pe=pe, ac=ac, cc=cc)
        plan = []  # (engine, fn)

        def emit(engine, fn, sem=None, inc=1, waits=()):
            """queue fn(eng); fn must emit exactly one instruction; waits
            emitted before it. Returns post-inc count of `sem`."""
            w = tuple(waits)

            def run(eng, fn=fn, w=w, sem=sem, inc=inc):
                for s, v in w:
                    if v > 0:
                        eng.wait_ge(S[s], v)
                ins = fn(eng)
                if sem is not None:
                    ins.then_inc(S[sem], inc)
            plan.append((engine, run))
            if sem is not None:
                C[sem] += inc
            return C[sem] if sem is not None else None

        mm_tiles = _chunks(SH, MMT)

        # ---------- input loads (sync) ----------
        loads = [
            (idxs[:, :], ext["idxall"][:, :]), (tab[0:64, 0, 1:TCOL], ext["xT"][0:64, :]),
        ]
        # xT goes into mst (it is [128, SH] f16) -- overlay
        loads = [
            (idxs[:, :], ext["idxall"][:, :]),
            (mst[:, :], ext["xT"][:, :]),
            (d64[:, :], ext["d64"][:, :]),
            (w1[:, :], ext["w1"][:, :]),
            (w2[:, :], ext["w2"][:, :]),
            (wst[:, :, :].rearrange("p a b -> p (a b)"), ext["wst"][:, :, :].rearrange("p a b -> p (a b)")),
            (wf[:, :], ext["wf"][:, :]),
            (b1[:, :], ext["b1"][:, :]),
            (b2[:, :], ext["b2"][:, :]),
            (bc[:, :], ext["bc"][:, :]),
            (bf[:, :], ext["bf"][:, :]),
            (alp[:, :], ext["alp"][:, :]),
        ]
        for dst_ap, src_ap in loads:
            emit("sync", lambda e, d=dst_ap, s=src_ap: e.dma_start(out=d, in_=s),
                 sem="sd", inc=16)
        in_done = C["sd"]

        # sentinel cols
        for w in range(4):
            emit("gpsimd", lambda e, w=w: e.memset(tab[:, w, 0:1], 0), sem="gp")

        # ---------- helpers ----------
        def mm_stage(lhs_ap_fn, rhs_buf, rhs_parts, out_parts, act_fn, bias_ap,
                     out_buf, scale_tiles, rhs_wait, alpha=0.0):
            """13-tile matmul -> (optional DVE psum scale) -> ACT -> out_buf.
            rhs_wait: list of (sem, count) gating the first MM tile."""
            bank_free = [("ve", 0), ("ve", 0)]  # last consumer of each bank
            for t, (off, n) in enumerate(mm_tiles):
                b = t % 2
                waits = list(rhs_wait if t == 0 else [])
                waits.append(bank_free[b])
                pe_c = emit(
                    "tensor",
                    lambda e, b=b, off=off, n=n, f=lhs_ap_fn: e.matmul(
                        ps[b][0:out_parts, 0:n], f(), rhs_buf[0:rhs_parts, off:off + n]),
                    sem="pe", waits=waits)
                if scale_tiles:
                    ve_c = emit(
                        "vector",
                        lambda e, b=b, off=off, n=n: e.tensor_tensor(
                            ps[b][0:out_parts, 0:n], ps[b][0:out_parts, 0:n],
                            d64[0:out_parts, off:off + n], OP.mult),
                        sem="ve", waits=[("pe", pe_c)])
                    act_wait = ("ve", ve_c)
                else:
                    act_wait = ("pe", pe_c)
                ac_c = emit(
                    "scalar",
                    lambda e, b=b, off=off, n=n: e.activation(
                        out_buf[0:out_parts, off:off + n], ps[b][0:out_parts, 0:n],
                        act_fn, bias=bias_ap, scale=1.0, alpha=alpha),
                    sem="ac", waits=[act_wait])
                bank_free[b] = ("ac", ac_c)
            return C["ac"]

        # ---------- FC stack ----------
        ac_fc1 = mm_stage(lambda: w1[:, :], mst, 128, 32, AF.Relu, b1[:, :],
                          h0, False, [("sd", in_done)])
        ac_fc2 = mm_stage(lambda: w2[:, :], h0, 32, 64, AF.Identity, b2[:, :],
                          hA, False, [("ac", ac_fc1)])
        pe_after_fc = C["pe"]
        emit("sync", lambda e: e.dma_start(out=dbg_fc[:, :], in_=hA[:, :]),
             sem="sd", inc=16, waits=[("ac", ac_fc2)])

        # ---------- comm + conv layers ----------
        h_map = [(hA, hB), (hB, hA), (hA, hB), (hB, gbuf)]
        last_send = 0
        last_cc = 0
        prev_merge_gp = 0
        last_gath_on_w = [C["gp"]] * 4   # gp count of last gather touching tab[w]
        mst_reader_pe = pe_after_fc      # pe count that frees mst for overwrite

        for li in range(4):
            h_in, h_out = h_map[li]
            # g = h_in * dis  (gbuf), then send + CC
            ve_g = emit("vector",
                        lambda e, h=h_in: e.tensor_tensor(
                            gbuf[:, :], h[:, :], d64[:, :], OP.mult),
                        sem="ve",
                        waits=[("ac", C["ac"]), ("sd", last_send), ("ve", C["ve"])])
            last_send = emit("sync",
                             lambda e: e.dma_start(out=ag_in[:, :], in_=gbuf[:, :]),
                             sem="sd", inc=16,
                             waits=[("ve", ve_g), ("cc", last_cc)])
            last_cc = emit("gpsimd",
                           lambda e: e.collective_compute(
                               "AllGather", OP.bypass,
                               replica_groups=[list(range(NCORES))],
                               ins=[ag_in[:, :].opt()],
                               outs=[ag_out[:, :, :].opt()]),
                           sem="cc", waits=[("sd", last_send)])
            # table loads
            tl = []
            for w in range(4):
                emit("sync",
                     lambda e, w=w: e.dma_start(out=tab[0:64, w, 1:SH + 1],
                                                in_=ag_out[w, :, :]),
                     sem="sd", inc=16,
                     waits=[("cc", last_cc), ("gp", last_gath_on_w[w])])
                c2 = emit("sync",
                          lambda e, w=w: e.dma_start(out=tab[64:128, w, 1:SH + 1],
                                                     in_=ag_out[w + 4, :, :]),
                          sem="sd", inc=16)
                tl.append(c2)

            # gathers per schedule
            ve_kadds = []            # ve counts of pack-adds (scr ring WAR)
            kadd_i = 0
            ve_wacc = [0, 0, 0, 0]   # ve count after last add touching acc[w]
            ve_mrg_master = 0
            mrg_done_gp = 0
            first_master = True
            cur_w = -1
            memset_done = set()
            for e_ in sched:
                kind, w, so_, sl = e_["kind"], e_["w"], e_["so"], e_["sl"]
                n = e_["n"]
                if w != cur_w:
                    cur_w = w
                if kind == "kpack":
                    if w not in memset_done:
                        memset_done.add(w)
                        mv = emit("vector",
                                  lambda e, w=w: e.memset(acc[:, w, :], 0),
                                  sem="ve",
                                  waits=[("gp", prev_merge_gp)])
                        ve_wacc[w] = mv
                    ring = kadd_i % 2
                    waits = [("sd", tl[w]), ("ve", ve_wacc[w])]
                    if kadd_i >= 2:
                        waits.append(("ve", ve_kadds[kadd_i - 2]))
                    g_c = emit("gpsimd",
                               lambda e, w=w, n=n, so_=so_, sl=sl, r=ring: e.indirect_copy(
                                   scr[:, r, 0:n], tab[:, w, :],
                                   idxs[:, so_:so_ + sl], True),
                               sem="gp", waits=waits)
                    last_gath_on_w[w] = g_c
                    pos_ = 0
                    v_c = 0
                    for (kk, soff, pn) in e_["pieces"]:
                        v_c = emit("vector",
                                   lambda e, w=w, soff=soff, pn=pn, p=pos_, r=ring: e.tensor_tensor(
                                       acc[:, w, soff:soff + pn],
                                       acc[:, w, soff:soff + pn],
                                       scr[:, r, p:p + pn], OP.add),
                                   sem="ve", waits=([("gp", g_c)] if pos_ == 0 else []))
                        pos_ += pn
                    ve_kadds.append(v_c)
                    ve_wacc[w] = v_c
                    kadd_i += 1
                else:  # merge
                    off = e_["off"]
                    target = mst if first_master else mrg
                    waits = [("ve", ve_wacc[w])]
                    if first_master and off == 0:
                        waits.append(("pe", mst_reader_pe))
                    if not first_master and off == 0:
                        waits.append(("ve", ve_mrg_master))
                    g_c = emit("gpsimd",
                               lambda e, w=w, off=off, n=n, so_=so_, sl=sl, t=target: e.indirect_copy(
                                   t[:, off:off + n], acc[:, w, :],
                                   idxs[:, so_:so_ + sl], True),
                               sem="gp", waits=waits)
                    mrg_done_gp = g_c
                    if off + n >= SH:  # window complete
                        if first_master:
                            first_master = False
                        else:
                            ve_mrg_master = emit(
                                "vector",
                                lambda e: e.tensor_tensor(
                                    mst[:, :], mst[:, :], mrg[:, :], OP.add),
                                sem="ve", waits=[("gp", g_c)])

            master_ready = [("ve", ve_mrg_master), ("gp", mrg_done_gp)]
            prev_merge_gp = mrg_done_gp

            # matmul + scale + act (+ residual)
            act_fn = AF.Identity if li == 3 else AF.Prelu
            bias_ap = bc[:, li:li + 1]
            if li == 3:
                # gbuf is the ACT target; ensure last send (g of li=2... ) done
                master_ready.append(("sd", last_send))
            if li == 0:
                emit("sync", lambda e: e.dma_start(out=dbg_mst[:, :], in_=mst[:, :]),
                     sem="sd", inc=16, waits=list(master_ready))
            mm_stage(lambda l=li: wst[:, l, :], mst, 128, 64, act_fn, bias_ap,
                     h_out, True, master_ready,
                     alpha=(0.0 if li == 3 else alp[:, :]))
            mst_reader_pe = C["pe"]
            if li == 0:
                emit("sync", lambda e: e.dma_start(out=dbg_o1[:, :], in_=hB[:, :]),
                     sem="sd", inc=16, waits=[("ac", C["ac"])])
            if li == 1:
                emit("vector", lambda e: e.tensor_tensor(
                    hA[:, :], hA[:, :], hB[:, :], OP.add),
                    sem="ve", waits=[("ac", C["ac"])])
            elif li == 3:
                emit("vector", lambda e: e.tensor_tensor(
                    gbuf[:, :], gbuf[:, :], hA[:, :], OP.add),
                    sem="ve", waits=[("ac", C["ac"])])

        # ---------- final projection ----------
        mm_stage(lambda: wf[:, :], gbuf, 64, 2, AF.Identity, bf[:, :],
                 h0, False, [("ve", C["ve"])])
        emit("sync", lambda e: e.dma_start(out=out_ext[:, :], in_=h0[0:2, :]),
             sem="sd", inc=16, waits=[("ac", C["ac"])])
        final_sd = C["sd"]

        # ---------- run plan per engine ----------
        by_eng = {"sync": [], "gpsimd": [], "vector": [], "tensor": [], "scalar": []}
        for eng_name, fn in plan:
            by_eng[eng_name].append(fn)

        @block.sync
        def _(eng):
            for fn in by_eng["sync"]:
                fn(eng)
            eng.wait_ge(sd, final_sd)

        @block.gpsimd
        def _(eng):
            for fn in by_eng["gpsimd"]:
                fn(eng)

        @block.vector
        def _(eng):
            for fn in by_eng["vector"]:
                fn(eng)

        @block.tensor
        def _(eng):
            for fn in by_eng["tensor"]:
                fn(eng)

        @block.scalar
        def _(eng):
            for fn in by_eng["scalar"]:
                fn(eng)

    return nc


def _device_forward(node_features, edge_index, fc1_W, fc1_b, fc2_W, fc2_b,
                    conv_Ws, conv_bs, final_W, final_b):
    from concourse.bass_utils import run_bass_kernel_spmd

    sched, nslot, blobs, dis = _preprocess(edge_index)
    nc = _build_bass(sched, nslot)

    x = np.zeros((NPAD, IN_CH), np.float16)
    x[:N_NODES] = np.asarray(node_features, np.float16)
    wst = np.zeros((128, 4, 64), np.float16)
    for l in range(4):
        W = np.asarray(conv_Ws[l], np.float16)
        wst[0:64, l, :] = W
        wst[64:128, l, :] = W
    bc = np.stack([np.asarray(b, np.float32) for b in conv_bs], axis=1)  # [64,4]

    in_maps = []
    for c in range(NCORES):
        lo = c * SH
        in_maps.append({
            "alp": np.full((64, 1), NEG_SLOPE, np.float32),
            "xT": np.ascontiguousarray(x[lo:lo + SH].T),
            "idxall": blobs[c],
            "d64": np.broadcast_to(dis[lo:lo + SH].astype(np.float16), (64, SH)).copy(),
            "w1": np.asarray(fc1_W, np.float16),
            "w2": np.asarray(fc2_W, np.float16),
            "wst": wst,
            "wf": np.asarray(final_W, np.float16),
            "b1": np.asarray(fc1_b, np.float32).reshape(32, 1),
            "b2": np.asarray(fc2_b, np.float32).reshape(64, 1),
            "bc": bc,
            "bf": np.asarray(final_b, np.float32).reshape(2, 1),
        })
    import os
    trace = bool(os.environ.get("BASS_TRACE"))
    res = run_bass_kernel_spmd(nc, in_maps, list(range(NCORES)), trace=trace)
    global LAST_EXEC_NS, LAST_RES
    LAST_EXEC_NS = res.exec_time_ns
    LAST_RES = res
    out = np.zeros((N_NODES, OUT_CH), np.float32)
    for c in range(NCORES):
        lo = c * SH
        hi = min(lo + SH, N_NODES)
        out[lo:hi] = res.results[c]["outp"][:, : hi - lo].astype(np.float32).T
    return out


def _host_forward(node_features, edge_index, fc1_W, fc1_b, fc2_W, fc2_b,
                  conv_Ws, conv_bs, final_W, final_b):
    x = np.asarray(node_features, np.float32)
    ei = np.asarray(edge_index)
    N = x.shape[0]
    loops = np.arange(N, dtype=np.int64)
    src = np.concatenate([ei[0].astype(np.int64), loops])
    dst = np.concatenate([ei[1].astype(np.int64), loops])
    deg = np.bincount(dst, minlength=N).astype(np.float32)
    dis = 1.0 / np.sqrt(np.maximum(deg, 1.0))
    norm = (dis[src] * dis[dst]).astype(np.float32)
    import scipy.sparse as sp
    Am = sp.csr_matrix((norm, (dst, src)), shape=(N, N), dtype=np.float32)

    def lrelu(v):
        return np.where(v >= 0, v, NEG_SLOPE * v).astype(np.float32)

    def gcn(h, W, b):
        return (Am @ (h)) @ W + b

    h = np.maximum(x @ fc1_W + fc1_b, 0.0) @ fc2_W + fc2_b
    out1 = lrelu(gcn(h, conv_Ws[0], conv_bs[0]))
    out2 = lrelu(gcn(out1, conv_Ws[1], conv_bs[1])) + out1
    out3 = lrelu(gcn(out2, conv_Ws[2], conv_bs[2]))
    out4 = gcn(out3, conv_Ws[3], conv_bs[3]) + out2
    return (out4 @ final_W + final_b).astype(np.float32)


def kernel(node_features, edge_index, fc1_W, fc1_b, fc2_W, fc2_b,
           conv1_W, conv1_b, conv2_W, conv2_b, conv3_W, conv3_b,
           conv4_W, conv4_b, final_W, final_b):
    conv_Ws = [np.asarray(w, np.float32) for w in (conv1_W, conv2_W, conv3_W, conv4_W)]
    conv_bs = [np.asarray(b, np.float32) for b in (conv1_b, conv2_b, conv3_b, conv4_b)]
    args = (node_features, edge_index, np.asarray(fc1_W, np.float32),
            np.asarray(fc1_b, np.float32), np.asarray(fc2_W, np.float32),
            np.asarray(fc2_b, np.float32), conv_Ws, conv_bs,
            np.asarray(final_W, np.float32), np.asarray(final_b, np.float32))
    try:
        return _device_forward(*args)
    except Exception as e:  # pragma: no cover - device fallback
        sys.stderr.write(f"device path failed ({type(e).__name__}: {e}); numpy fallback\n")
        return _host_forward(*args)


# revision 15
# speedup vs baseline: 1.2245x; 1.2245x over previous
"""GCN (4-layer + MLP head/tail) on 8 TRN2 NeuronCores.

Design:
  - Nodes padded to 50008 = 8 shards of SH=6251; core c owns dst shard c.
  - Node features for gathers live in 4 SBUF "window" tables [128, 6252] f16:
    window w rows 0-63 = shard w nodes (1 feat/partition), rows 64-127 =
    shard w+4.  Col 0 of each window is a zero sentinel.
  - Aggregation = per-(window,stream) k-step prefix gathers (indirect_copy,
    the one natively supported SBUF gather; table <= 16KB/partition) into
    per-window accumulators, degree-sorted so step k covers a prefix.
  - Per-window merge = one permuting indirect_copy back to canonical order,
    summed into a [128, SH] master (top rows = shards 0-3 sums, bottom =
    shards 4-7).
  - The cross-half fold AND the GCN weight multiply fuse into one matmul
    with vertically stacked weights [W;W] (K=128).
  - dst-side deg_isqrt scaling commutes with the matmul -> applied per
    column on PSUM tiles (DVE), then ACT does bias+LeakyReLU.
  - Inter-layer exchange: DMA g-slice to DRAM + ncfw CC AllGather.
Falls back to a numpy implementation if the device path fails.
"""
import sys

sys.path.insert(0, "/opt/trn_rl_repo")

import numpy as np

N_NODES = 50000
N_EDGES = 800000
IN_CH = 128
HID = 64
OUT_CH = 2
NEG_SLOPE = 0.2
NCORES = 8
LAST_EXEC_NS = None
LAST_RES = None
SH = 6272                      # nodes per shard, mult of 32 (8*6272 = 50176)
NPAD = NCORES * SH
TCOL = SH + 2                  # table cols (even): sentinel col 0, nodes 1..SH
CH = 1024                      # indirect_copy output cap (elems/partition)
MMT = 512                      # matmul moving-dim tile


def _chunks(n, c=CH):
    out = []
    off = 0
    while off < n:
        out.append((off, min(c, n - off)))
        off += c
    return out


def _wrap2(top, bot, slots):
    """top/bot: 1-D uint16 lists (len <= slots*16) -> [128, slots] wrapped:
    groups 0-3 (partitions 0-63) hold `top`, groups 4-7 hold `bot`."""
    blk = np.zeros((128, slots), np.uint16)
    for arr, g0 in ((top, 0), (bot, 4)):
        a = np.zeros(slots * 16, np.uint16)
        a[: len(arr)] = arr
        a = a.reshape(slots, 16).T  # [16, slots]
        for g in range(g0, g0 + 4):
            blk[16 * g: 16 * g + 16] = a
    return blk


def _preprocess(edge_index):
    """Build the uniform gather schedule + per-core index blobs."""
    ei = np.asarray(edge_index)
    src = np.concatenate([ei[0].astype(np.int64), np.arange(N_NODES, dtype=np.int64)])
    dst = np.concatenate([ei[1].astype(np.int64), np.arange(N_NODES, dtype=np.int64)])
    deg = np.bincount(dst, minlength=N_NODES).astype(np.float32)
    dis = np.zeros(NPAD, np.float32)
    dis[:N_NODES] = 1.0 / np.sqrt(np.maximum(deg, 1.0))

    owner = dst // SH
    bkt = src // SH
    dloc = (dst - owner * SH).astype(np.int64)
    scol = (src - bkt * SH + 1).astype(np.uint16)  # 1-based; 0 = sentinel

    order = np.lexsort((dloc, bkt, owner))
    so, sb, sdl, scl = owner[order], bkt[order], dloc[order], scol[order]
    key = (so * 8 + sb) * SH + sdl
    starts = np.r_[True, np.diff(key) != 0]
    run_start = np.flatnonzero(starts)
    run_id = np.cumsum(starts) - 1
    k_rank = np.arange(len(key)) - run_start[run_id]
    lens = np.diff(np.r_[run_start, len(key)])

    degob = np.zeros(8 * 8 * SH, np.int32)
    degob[key[run_start]] = lens
    degob = degob.reshape(8, 8, SH)

    # per (core o, bucket b): degree-sorted order, inverse positions, A matrix
    pos = np.zeros((8, 8, SH), np.uint16)
    A = {}
    maxk = np.zeros((8, 8), np.int32)
    for o in range(8):
        for b in range(8):
            d = degob[o, b]
            pi = np.argsort(-d, kind="stable")
            p = np.empty(SH, np.uint16)
            p[pi] = np.arange(SH, dtype=np.uint16)
            pos[o, b] = p
            mk = int(d.max())
            maxk[o, b] = mk
            A[(o, b)] = np.zeros((max(mk, 1), SH), np.uint16)
    for o in range(8):
        mo = so == o
        for b in range(8):
            m = mo & (sb == b)
            A[(o, b)][k_rank[m], pos[o, b][sdl[m]]] = scl[m]

    # prefix counts n[o,b,k] = #{deg_ob > k}
    nob = {}
    for o in range(8):
        for b in range(8):
            d = degob[o, b]
            nob[(o, b)] = np.array([(d > k).sum() for k in range(maxk[o, b])],
                                   np.int32)

    # uniform schedule: list of sub-instructions
    # entry: dict(kind, w, off, n, so(slot off), sl(slots), k)
    sched = []
    slot_off = 0

    def add(kind, w, off, n, k=0):
        nonlocal slot_off
        sl = (n + 15) // 16
        sched.append(dict(kind=kind, w=w, off=off, n=n, k=k,
                          so=slot_off, sl=sl))
        slot_off += sl

    for w in range(4):
        kmax = 1
        for o in range(8):
            kmax = max(kmax, maxk[o, w], maxk[o, w + 4])
        for off, n in _chunks(SH):
            add("init", w, off, n, 0)
        for k in range(1, kmax):
            nk = 0
            for o in range(8):
                for b in (w, w + 4):
                    nl = nob[(o, b)]
                    if k < len(nl):
                        nk = max(nk, int(nl[k]))
            if nk == 0:
                continue
            nk = min((nk + 31) // 32 * 32, SH)
            for off, n in _chunks(nk):
                add("kadd", w, off, n, k)
        for off, n in _chunks(SH):
            add("merge", w, off, n, 0)
    nslot = slot_off

    # per-core index blobs
    blobs = []
    for o in range(8):
        blob = np.zeros((128, nslot), np.uint16)
        for e in sched:
            w, off, n, k = e["w"], e["off"], e["n"], e["k"]
            if e["kind"] in ("init", "kadd"):
                rows = []
                for b in (w, w + 4):
                    Ao = A[(o, b)]
                    if k < Ao.shape[0]:
                        rows.append(Ao[k, off:off + n])
                    else:
                        rows.append(np.zeros(n, np.uint16))
                blk = _wrap2(rows[0], rows[1], e["sl"])
            else:  # merge
                blk = _wrap2(pos[o, w][off:off + n],
                             pos[o, w + 4][off:off + n], e["sl"])
            blob[:, e["so"]: e["so"] + e["sl"]] = blk
        blobs.append(blob)

    return sched, nslot, blobs, dis


def _build_bass(sched, nslot):
    import concourse.bass as bass
    import concourse.mybir as mybir

    F16 = mybir.dt.float16
    F32 = mybir.dt.float32
    U16 = mybir.dt.uint16
    OP = mybir.AluOpType
    AF = mybir.ActivationFunctionType

    nc = bass.Bass(num_devices=NCORES)

    ext = {}
    for name, shape, dt in [
        ("xT", [128, SH], F16), ("idxall", [128, nslot], U16),
        ("d64", [64, SH], F16),
        ("w1", [128, 32], F16), ("w2", [32, 64], F16),
        ("wst", [128, 4, 64], F16), ("wf", [64, 2], F16),
        ("b1", [32, 1], F32), ("b2", [64, 1], F32),
        ("bc", [64, 4], F32), ("bf", [2, 1], F32), ("alp", [64, 1], F32),
    ]:
        ext[name] = nc.declare_dram_parameter(name, shape, dt, isOutput=False)
    out_ext = nc.declare_dram_parameter("outp", [2, SH], F16, isOutput=True)
    dbg_fc = nc.declare_dram_parameter("dbg_fc", [64, SH], F16, isOutput=True)
    dbg_mst = nc.declare_dram_parameter("dbg_mst", [128, SH], F16, isOutput=True)
    dbg_o1 = nc.declare_dram_parameter("dbg_o1", [64, SH], F16, isOutput=True)
    ag_in = nc.dram_tensor("ag_in", [64, SH], F16)
    ag_out = nc.dram_tensor("ag_out", [8, 64, SH], F16)

    from contextlib import ExitStack
    with ExitStack() as _st:
        def _sb(name, shape, dt):
            return _st.enter_context(nc.sbuf_tensor(name, shape, dt))

        idxs = _sb("idx_s", [128, nslot], U16)
        tab = _sb("tab_s", [128, 4, TCOL], F16)
        acc = _sb("acc_s", [128, 4, SH], F16)
        scr = _sb("scr_s", [128, 2, CH], F16)
        mrg = _sb("mrg_s", [128, SH], F16)
        mst = _sb("mst_s", [128, SH], F16)
        hA = _sb("hA_s", [64, SH], F16)
        hB = _sb("hB_s", [64, SH], F16)
        h0 = _sb("h0_s", [32, SH], F16)
        gbuf = _sb("g_s", [64, SH], F16)
        d64 = _sb("d64_s", [64, SH], F16)
        w1 = _sb("w1_s", [128, 32], F16)
        w2 = _sb("w2_s", [32, 64], F16)
        wst = _sb("wst_s", [128, 4, 64], F16)
        wf = _sb("wf_s", [64, 2], F16)
        b1 = _sb("b1_s", [32, 1], F32)
        b2 = _sb("b2_s", [64, 1], F32)
        bc = _sb("bc_s", [64, 4], F32)
        bf = _sb("bf_s", [2, 1], F32)
        alp = _sb("alp_s", [64, 1], F32)
        ps0 = _st.enter_context(nc.psum_tensor("ps0", [64, MMT], F32))
        ps1 = _st.enter_context(nc.psum_tensor("ps1", [64, MMT], F32))
        sd = _st.enter_context(nc.semaphore("sd"))
        gp = _st.enter_context(nc.semaphore("gp"))
        ve = _st.enter_context(nc.semaphore("ve"))
        pe = _st.enter_context(nc.semaphore("pe"))
        ac = _st.enter_context(nc.semaphore("ac"))
        cc = _st.enter_context(nc.semaphore("cc"))
        block = _st.enter_context(nc.Block())

        ps = [ps0, ps1]
        C = dict(sd=0, gp=0, ve=0, pe=0, ac=0, cc=0)
        S = dict(sd=sd, gp=gp, ve=ve, pe=pe, ac=ac, cc=cc)
        plan = []  # (engine, fn)

        def emit(engine, fn, sem=None, inc=1, waits=()):
            """queue fn(eng); fn must emit exactly one instruction; waits
            emitted before it. Returns post-inc count of `sem`."""
            w = tuple(waits)

            def run(eng, fn=fn, w=w, sem=sem, inc=inc):
                for s, v in w:
                    if v > 0:
                        eng.wait_ge(S[s], v)
                ins = fn(eng)
                if sem is not None:
                    ins.then_inc(S[sem], inc)
            plan.append((engine, run))
            if sem is not None:
                C[sem] += inc
            return C[sem] if sem is not None else None

        mm_tiles = _chunks(SH, MMT)

        # ---------- input loads (sync) ----------
        loads = [
            (idxs[:, :], ext["idxall"][:, :]), (tab[0:64, 0, 1:TCOL], ext["xT"][0:64, :]),
        ]
        # xT goes into mst (it is [128, SH] f16) -- overlay
        loads = [
            (idxs[:, :], ext["idxall"][:, :]),
            (mst[:, :], ext["xT"][:, :]),
            (d64[:, :], ext["d64"][:, :]),
            (w1[:, :], ext["w1"][:, :]),
            (w2[:, :], ext["w2"][:, :]),
            (wst[:, :, :].rearrange("p a b -> p (a b)"), ext["wst"][:, :, :].rearrange("p a b -> p (a b)")),
            (wf[:, :], ext["wf"][:, :]),
            (b1[:, :], ext["b1"][:, :]),
            (b2[:, :], ext["b2"][:, :]),
            (bc[:, :], ext["bc"][:, :]),
            (bf[:, :], ext["bf"][:, :]),
            (alp[:, :], ext["alp"][:, :]),
        ]
        for dst_ap, src_ap in loads:
            emit("sync", lambda e, d=dst_ap, s=src_ap: e.dma_start(out=d, in_=s),
                 sem="sd", inc=16)
        in_done = C["sd"]

        # sentinel cols
        for w in range(4):
            emit("gpsimd", lambda e, w=w: e.memset(tab[:, w, 0:1], 0), sem="gp")

        # ---------- helpers ----------
        def mm_stage(lhs_ap_fn, rhs_buf, rhs_parts, out_parts, act_fn, bias_ap,
                     out_buf, scale_tiles, rhs_wait, alpha=0.0):
            """13-tile matmul -> (optional DVE psum scale) -> ACT -> out_buf.
            rhs_wait: list of (sem, count) gating the first MM tile."""
            bank_free = [("ve", 0), ("ve", 0)]  # last consumer of each bank
            for t, (off, n) in enumerate(mm_tiles):
                b = t % 2
                waits = list(rhs_wait if t == 0 else [])
                waits.append(bank_free[b])
                pe_c = emit(
                    "tensor",
                    lambda e, b=b, off=off, n=n, f=lhs_ap_fn: e.matmul(
                        ps[b][0:out_parts, 0:n], f(), rhs_buf[0:rhs_parts, off:off + n]),
                    sem="pe", waits=waits)
                if scale_tiles:
                    ve_c = emit(
                        "vector",
                        lambda e, b=b, off=off, n=n: e.tensor_tensor(
                            ps[b][0:out_parts, 0:n], ps[b][0:out_parts, 0:n],
                            d64[0:out_parts, off:off + n], OP.mult),
                        sem="ve", waits=[("pe", pe_c)])
                    act_wait = ("ve", ve_c)
                else:
                    act_wait = ("pe", pe_c)
                ac_c = emit(
                    "scalar",
                    lambda e, b=b, off=off, n=n: e.activation(
                        out_buf[0:out_parts, off:off + n], ps[b][0:out_parts, 0:n],
                        act_fn, bias=bias_ap, scale=1.0, alpha=alpha),
                    sem="ac", waits=[act_wait])
                bank_free[b] = ("ac", ac_c)
            return C["ac"]

        # ---------- FC stack ----------
        ac_fc1 = mm_stage(lambda: w1[:, :], mst, 128, 32, AF.Relu, b1[:, :],
                          h0, False, [("sd", in_done)])
        ac_fc2 = mm_stage(lambda: w2[:, :], h0, 32, 64, AF.Identity, b2[:, :],
                          hA, False, [("ac", ac_fc1)])
        pe_after_fc = C["pe"]
        emit("sync", lambda e: e.dma_start(out=dbg_fc[:, :], in_=hA[:, :]),
             sem="sd", inc=16, waits=[("ac", ac_fc2)])

        # ---------- comm + conv layers ----------
        h_map = [(hA, hB), (hB, hA), (hA, hB), (hB, gbuf)]
        last_send = 0
        last_cc = 0
        last_gath_on_w = [C["gp"]] * 4   # gp count of last gather touching tab[w]
        mst_reader_pe = pe_after_fc      # pe count that frees mst for overwrite

        for li in range(4):
            h_in, h_out = h_map[li]
            # g = h_in * dis  (gbuf), then send + CC
            ve_g = emit("vector",
                        lambda e, h=h_in: e.tensor_tensor(
                            gbuf[:, :], h[:, :], d64[:, :], OP.mult),
                        sem="ve",
                        waits=[("ac", C["ac"]), ("sd", last_send), ("ve", C["ve"])])
            last_send = emit("sync",
                             lambda e: e.dma_start(out=ag_in[:, :], in_=gbuf[:, :]),
                             sem="sd", inc=16,
                             waits=[("ve", ve_g), ("cc", last_cc)])
            last_cc = emit("gpsimd",
                           lambda e: e.collective_compute(
                               "AllGather", OP.bypass,
                               replica_groups=[list(range(NCORES))],
                               ins=[ag_in[:, :].opt()],
                               outs=[ag_out[:, :, :].opt()]),
                           sem="cc", waits=[("sd", last_send)])
            # table loads
            tl = []
            for w in range(4):
                emit("sync",
                     lambda e, w=w: e.dma_start(out=tab[0:64, w, 1:SH + 1],
                                                in_=ag_out[w, :, :]),
                     sem="sd", inc=16,
                     waits=[("cc", last_cc), ("gp", last_gath_on_w[w])])
                c2 = emit("sync",
                          lambda e, w=w: e.dma_start(out=tab[64:128, w, 1:SH + 1],
                                                     in_=ag_out[w + 4, :, :]),
                          sem="sd", inc=16)
                tl.append(c2)

            # gathers per schedule
            ve_kadds = []            # ve counts of kadds (for scr ring WAR)
            kadd_i = 0
            ve_wacc = [0, 0, 0, 0]   # ve count after last kadd touching acc[w]
            ve_mrg_master = 0
            mrg_done_gp = 0
            first_master = True
            cur_w = -1
            for e_ in sched:
                kind, w, off, n, so_, sl = (e_["kind"], e_["w"], e_["off"],
                                            e_["n"], e_["so"], e_["sl"])
                if w != cur_w:
                    cur_w = w
                if kind == "init":
                    g_c = emit("gpsimd",
                               lambda e, w=w, off=off, n=n, so_=so_, sl=sl: e.indirect_copy(
                                   acc[:, w, off:off + n], tab[:, w, :],
                                   idxs[:, so_:so_ + sl], True),
                               sem="gp",
                               waits=[("sd", tl[w]), ("ve", ve_wacc[w])])
                    last_gath_on_w[w] = g_c
                elif kind == "kadd":
                    ring = kadd_i % 2
                    waits = [("sd", tl[w])]
                    if kadd_i >= 2:
                        waits.append(("ve", ve_kadds[kadd_i - 2]))
                    g_c = emit("gpsimd",
                               lambda e, w=w, n=n, so_=so_, sl=sl, r=ring: e.indirect_copy(
                                   scr[:, r, 0:n], tab[:, w, :],
                                   idxs[:, so_:so_ + sl], True),
                               sem="gp", waits=waits)
                    last_gath_on_w[w] = g_c
                    v_c = emit("vector",
                               lambda e, w=w, off=off, n=n, r=ring: e.tensor_tensor(
                                   acc[:, w, off:off + n], acc[:, w, off:off + n],
                                   scr[:, r, 0:n], OP.add),
                               sem="ve", waits=[("gp", g_c)])
                    ve_kadds.append(v_c)
                    ve_wacc[w] = v_c
                    kadd_i += 1
                else:  # merge
                    target = mst if first_master else mrg
                    waits = [("ve", ve_wacc[w])]
                    if first_master and off == 0:
                        waits.append(("pe", mst_reader_pe))
                    if not first_master and off == 0:
                        waits.append(("ve", ve_mrg_master))
                    g_c = emit("gpsimd",
                               lambda e, w=w, off=off, n=n, so_=so_, sl=sl, t=target: e.indirect_copy(
                                   t[:, off:off + n], acc[:, w, :],
                                   idxs[:, so_:so_ + sl], True),
                               sem="gp", waits=waits)
                    mrg_done_gp = g_c
                    if off + n >= SH:  # window complete
                        if first_master:
                            first_master = False
                        else:
                            ve_mrg_master = emit(
                                "vector",
                                lambda e: e.tensor_tensor(
                                    mst[:, :], mst[:, :], mrg[:, :], OP.add),
                                sem="ve", waits=[("gp", g_c)])

            master_ready = [("ve", ve_mrg_master), ("gp", mrg_done_gp)]

            # matmul + scale + act (+ residual)
            act_fn = AF.Identity if li == 3 else AF.Prelu
            bias_ap = bc[:, li:li + 1]
            if li == 3:
                # gbuf is the ACT target; ensure last send (g of li=2... ) done
                master_ready.append(("sd", last_send))
            if li == 0:
                emit("sync", lambda e: e.dma_start(out=dbg_mst[:, :], in_=mst[:, :]),
                     sem="sd", inc=16, waits=list(master_ready))
            mm_stage(lambda l=li: wst[:, l, :], mst, 128, 64, act_fn, bias_ap,
                     h_out, True, master_ready,
                     alpha=(0.0 if li == 3 else alp[:, :]))
            mst_reader_pe = C["pe"]
            if li == 0:
                emit("sync", lambda e: e.dma_start(out=dbg_o1[:, :], in_=hB[:, :]),
                     sem="sd", inc=16, waits=[("ac", C["ac"])])
            if li == 1:
                emit("vector", lambda e: e.tensor_tensor(
                    hA[:, :], hA[:, :], hB[:, :], OP.add),
                    sem="ve", waits=[("ac", C["ac"])])
            elif li == 3:
                emit("vector", lambda e: e.tensor_tensor(
                    gbuf[:, :], gbuf[:, :], hA[:, :], OP.add),
                    sem="ve", waits=[("ac", C["ac"])])

        # ---------- final projection ----------
        mm_stage(lambda: wf[:, :], gbuf, 64, 2, AF.Identity, bf[:, :],
                 h0, False, [("ve", C["ve"])])
        emit("sync", lambda e: e.dma_start(out=out_ext[:, :], in_=h0[0:2, :]),
             sem="sd", inc=16, waits=[("ac", C["ac"])])
        final_sd = C["sd"]

        # ---------- run plan per engine ----------
        by_eng = {"sync": [], "gpsimd": [], "vector": [], "tensor": [], "scalar": []}
        for eng_name, fn in plan:
            by_eng[eng_name].append(fn)

        @block.sync
        def _(eng):
            for fn in by_eng["sync"]:
                fn(eng)
            eng.wait_ge(sd, final_sd)

        @block.gpsimd
        def _(eng):
            for fn in by_eng["gpsimd"]:
                fn(eng)

        @block.vector
        def _(eng):
            for fn in by_eng["vector"]:
                fn(eng)

        @block.tensor
        def _(eng):
            for fn in by_eng["tensor"]:
                fn(eng)

        @block.scalar
        def _(eng):
            for fn in by_eng["scalar"]:
                fn(eng)

    return nc


def _device_forward(node_features, edge_index, fc1_W, fc1_b, fc2_W, fc2_b,
                    conv_Ws, conv_bs, final_W, final_b):
    from concourse.bass_utils import run_bass_kernel_spmd

    sched, nslot, blobs, dis = _preprocess(edge_index)
    nc = _build_bass(sched, nslot)

    x = np.zeros((NPAD, IN_CH), np.float16)
    x[:N_NODES] = np.asarray(node_features, np.float16)
    wst = np.zeros((128, 4, 64), np.float16)
    for l in range(4):
        W = np.asarray(conv_Ws[l], np.float16)
        wst[0:64, l, :] = W
        wst[64:128, l, :] = W
    bc = np.stack([np.asarray(b, np.float32) for b in conv_bs], axis=1)  # [64,4]

    in_maps = []
    for c in range(NCORES):
        lo = c * SH
        in_maps.append({
            "alp": np.full((64, 1), NEG_SLOPE, np.float32),
            "xT": np.ascontiguousarray(x[lo:lo + SH].T),
            "idxall": blobs[c],
            "d64": np.broadcast_to(dis[lo:lo + SH].astype(np.float16), (64, SH)).copy(),
            "w1": np.asarray(fc1_W, np.float16),
            "w2": np.asarray(fc2_W, np.float16),
            "wst": wst,
            "wf": np.asarray(final_W, np.float16),
            "b1": np.asarray(fc1_b, np.float32).reshape(32, 1),
            "b2": np.asarray(fc2_b, np.float32).reshape(64, 1),
            "bc": bc,
            "bf": np.asarray(final_b, np.float32).reshape(2, 1),
        })
    import os
    trace = bool(os.environ.get("BASS_TRACE"))
    res = run_bass_kernel_spmd(nc, in_maps, list(range(NCORES)), trace=trace)
    global LAST_EXEC_NS, LAST_RES
    LAST_EXEC_NS = res.exec_time_ns
    LAST_RES = res
    out = np.zeros((N_NODES, OUT_CH), np.float32)
    for c in range(NCORES):
        lo = c * SH
        hi = min(lo + SH, N_NODES)
        out[lo:hi] = res.results[c]["outp"][:, : hi - lo].astype(np.float32).T
    return out


def _host_forward(node_features, edge_index, fc1_W, fc1_b, fc2_W, fc2_b,
                  conv_Ws, conv_bs, final_W, final_b):
    x = np.asarray(node_features, np.float32)
    ei = np.asarray(edge_index)
    N = x.shape[0]
    loops = np.arange(N, dtype=np.int64)
    src = np.concatenate([ei[0].astype(np.int64), loops])
    dst = np.concatenate([ei[1].astype(np.int64), loops])
    deg = np.bincount(dst, minlength=N).astype(np.float32)
    dis = 1.0 / np.sqrt(np.maximum(deg, 1.0))
    norm = (dis[src] * dis[dst]).astype(np.float32)
    import scipy.sparse as sp
    Am = sp.csr_matrix((norm, (dst, src)), shape=(N, N), dtype=np.float32)

    def lrelu(v):
        return np.where(v >= 0, v, NEG_SLOPE * v).astype(np.float32)

    def gcn(h, W, b):
        return (Am @ (h)) @ W + b

    h = np.maximum(x @ fc1_W + fc1_b, 0.0) @ fc2_W + fc2_b
    out1 = lrelu(gcn(h, conv_Ws[0], conv_bs[0]))
    out2 = lrelu(gcn(out1, conv_Ws[1], conv_bs[1])) + out1
    out3 = lrelu(gcn(out2, conv_Ws[2], conv_bs[2]))
    out4 = gcn(out3, conv_Ws[3], conv_bs[3]) + out2
    return (out4 @ final_W + final_b).astype(np.float32)


def kernel(node_features, edge_index, fc1_W, fc1_b, fc2_W, fc2_b,
           conv1_W, conv1_b, conv2_W, conv2_b, conv3_W, conv3_b,
           conv4_W, conv4_b, final_W, final_b):
    conv_Ws = [np.asarray(w, np.float32) for w in (conv1_W, conv2_W, conv3_W, conv4_W)]
    conv_bs = [np.asarray(b, np.float32) for b in (conv1_b, conv2_b, conv3_b, conv4_b)]
    args = (node_features, edge_index, np.asarray(fc1_W, np.float32),
            np.asarray(fc1_b, np.float32), np.asarray(fc2_W, np.float32),
            np.asarray(fc2_b, np.float32), conv_Ws, conv_bs,
            np.asarray(final_W, np.float32), np.asarray(final_b, np.float32))
    try:
        return _device_forward(*args)
    except Exception as e:  # pragma: no cover - device fallback
        sys.stderr.write(f"device path failed ({type(e).__name__}: {e}); numpy fallback\n")
        return _host_forward(*args)
